# revision 32
# baseline (speedup 1.0000x reference)
"""LCA layer kernel for Trainium2, data-parallel over tokens on 8 NeuronCores.

Reference computation (per token row x of d_model=1024, W [1024, 4096]):
    b = x @ W;  G = W^T W with zero diag;  u_0 = 0
    10x: a = relu(u - 0.1); u = 0.9 u + 0.1 (b - a @ G)
    out = relu(u - 0.1) @ W^T

Device algorithm (per core, 1024 tokens = 2 blocks of T=512, all in SBUF):
  * factor a@G = (a @ W^T) @ W - g * a  with g = diag(W^T W); fold dt/tau
    into W01 = 0.1 W so B' = 0.1 b and the update is
        u' = 0.9 u + B' - (a W^T) W01 + g2*a,   g2 = 0.1 g.
  * u1 = B' exactly (u0=0); a1/a2 are ~94/80% zero, so their G-terms are
    dropped: u3 = 2.71 B' free init, then 7 iterated steps instead of 9
    (numerically validated, ~1e-3 of the error budget).
  * the 7 iterated steps run entirely in fp8e4 with DoubleRow matmuls
    (2 fp8 MACs/PE/cycle, 0.5 cyc/row): a8 = fp8(s_a relu(u-.1)),
    ht8 = fp8(s_h a W^T); the y-psum accumulates w018 x ht8 PLUS a
    diagonal fp8 stationary pair that applies the +g2*a correction
    inside the same matmul group (no elementwise op needed for it).
  * u, B' stored bf16; per 2-chunk pair the update is spread over three
    engines (scalar_tensor_tensor runs 1x on DVE only, tensor_scalar
    gets the 4x mode, plain tensor adds are legal on GPSIMD):
        q = bf16(-py/(s_w01 s_h) + bp)   DVE stt (only psum-capable)
        v = bf16(0.9 u)                  DVE ts-mul / ACT copy (by parity)
        u' = v + q                       GPSIMD / DVE tensor add
        a8 = fp8(relu(s_a u' - .1 s_a))  ACT, in quads; pairs at the
                                         step tail to unblock next step
  * first (B' = x@W01) and last (out = a@W^T) matmuls stay bf16 for
    accuracy; their bf16 weights are streamed from HBM in 16KB/partition
    pieces and never kept resident (SBUF holds fp8 weights + state).
  Emulated end-to-end error vs fp32 reference: rel_l2 ~ 1.06e-2
  (measured on hardware: 1.02e-2; gate is 2e-2).
"""

import numpy as np
import ml_dtypes

P = 128          # partitions
T = 512          # tokens per block
NBLK = 2         # blocks per core (2*512 = 1024 tokens/core)
NSTEPS = 7       # iterated fp8 steps (u4..u10); u3 = 2.71 B' is free
C_INIT = 1.0 + 0.9 + 0.81   # u3 = 2.71 B' (a1@G, a2@G dropped -- validated)
DM = 1024        # d_model
DL = 4096        # d_lca
NDM = DM // P    # 8 d_model chunks
NDL = DL // P    # 32 d_lca chunks
NCORES = 8
TOK_CORE = NBLK * T

S_W = 1024.0     # fp8 scale for W^T (ht matmul stationary)
S_W01 = 4096.0   # fp8 scale for 0.1*W (y matmul stationary)
S_A = 16.0       # fp8 scale for a
S_H = 8.0        # fp8 scale for ht
INV_SY = 1.0 / (S_W01 * S_H)

BF16 = ml_dtypes.bfloat16
F8 = ml_dtypes.float8_e4m3    # TRN fp8e4 (max +-240)

_CACHE = {}

TRACE = False
V_ACT = (2, 3)  # jp%4 values whose v-op runs on ACT
TT_DVE = (1, 3)  # jp%4 values whose final add runs on DVE
LAST_RESULT = None


def apply_reduced():
    """Shrink work (same tensor ranks/layouts) for overhead calibration."""
    global NSTEPS, NBLK, TOK_CORE
    NSTEPS = 1
    NBLK = 1
    TOK_CORE = NBLK * T


def _build_nc():
    import concourse.bacc as bacc
    import concourse.tile as tile
    import concourse.mybir as mybir

    dt = mybir.dt
    Alu = mybir.AluOpType
    Act = mybir.ActivationFunctionType
    DR = mybir.MatmulPerfMode.DoubleRow

    nc = bacc.Bacc("TRN2", target_bir_lowering=False, debug=False,
                   num_devices=NCORES)

    xt_d = nc.dram_tensor("xt", [NBLK, P, NDM, T], dt.bfloat16,
                          kind="ExternalInput").ap()
    wt8_d = nc.dram_tensor("wt8", [P, NDL, DM], dt.float8e4,
                           kind="ExternalInput").ap()
    w018_d = nc.dram_tensor("w018", [P, NDM, DL], dt.float8e4,
                            kind="ExternalInput").ap()
    dd8_d = nc.dram_tensor("dd8", [P, 2 * NDL, P], dt.float8e4,
                           kind="ExternalInput").ap()
    w01bf_d = nc.dram_tensor("w01bf", [P, NDM, DL], dt.bfloat16,
                             kind="ExternalInput").ap()
    wtbf_d = nc.dram_tensor("wtbf", [P, NDL, DM], dt.bfloat16,
                            kind="ExternalInput").ap()
    out_d = nc.dram_tensor("out", [TOK_CORE, DM], dt.float32,
                           kind="ExternalOutput").ap()

    with tile.TileContext(nc) as tc:
        with (
            tc.tile_pool(name="wpool", bufs=1) as wpool,
            tc.tile_pool(name="state", bufs=1) as state,
            tc.tile_pool(name="htp", bufs=2) as htp,
            tc.tile_pool(name="xio", bufs=1) as xio,
            tc.tile_pool(name="wstr", bufs=2) as wstr,
            tc.tile_pool(name="oio", bufs=1) as oio,
            tc.tile_pool(name="tmp", bufs=4) as tmp,
            tc.tile_pool(name="psum", bufs=4, space="PSUM") as psum,
        ):
            # ---- resident fp8 weights ----
            wt8 = wpool.tile([P, NDL, DM], dt.float8e4, tag="wt8")
            w018 = wpool.tile([P, NDM, DL], dt.float8e4, tag="w018")
            dd8 = wpool.tile([P, 2 * NDL, P], dt.float8e4, tag="dd8")
            nlam_a = wpool.tile([P, 1], dt.float32, tag="nlam_a")
            nlam = wpool.tile([P, 1], dt.float32, tag="nlam")
            nc.gpsimd.memset(nlam_a[:], -0.1 * S_A)
            nc.gpsimd.memset(nlam[:], -0.1)

            first_dma_issued = False

            for blk in range(NBLK):
                xt = xio.tile([P, NDM, T], dt.bfloat16, tag="xt")
                nc.sync.dma_start(xt[:], xt_d[blk])

                u = state.tile([P, NDL, T], dt.bfloat16, tag="u")
                bp = state.tile([P, NDL, T], dt.bfloat16, tag="bp")
                a8 = state.tile([P, NDL + 1, T], dt.float8e4, tag="a8")
                if blk == 0:
                    nc.gpsimd.memset(a8[:, NDL, :], 0.0)

                # ---- phase A: B' = x @ W01 (bf16), u=1.9B', bp=B' ----
                for jhg in range(DL // T):
                    ws = wstr.tile([P, NDM, T], dt.bfloat16, tag="ws")
                    nc.sync.dma_start(
                        ws[:], w01bf_d[:, :, jhg * T:(jhg + 1) * T])
                    if not first_dma_issued:
                        # resident weights after the first A-piece so the
                        # pipeline head isn't DMA-starved
                        first_dma_issued = True
                        for kg in range(0, NDL, 8):
                            nc.sync.dma_start(wt8[:, kg:kg + 8, :],
                                              wt8_d[:, kg:kg + 8, :])
                        for mg in range(0, NDM, 2):
                            nc.sync.dma_start(w018[:, mg:mg + 2, :],
                                              w018_d[:, mg:mg + 2, :])
                        nc.sync.dma_start(dd8[:], dd8_d[:])
                    for jp in range(T // (2 * P)):
                        jc0 = jhg * (T // P) + jp * 2
                        pb2 = psum.tile([P, 2, T], dt.float32, tag="mm")
                        for j2 in range(2):
                            jl = jp * 2 + j2
                            for dmc in range(NDM):
                                nc.tensor.matmul(
                                    pb2[:, j2, :],
                                    ws[:, dmc, jl * P:(jl + 1) * P],
                                    xt[:, dmc, :],
                                    start=(dmc == 0), stop=(dmc == NDM - 1))
                        nc.scalar.activation(u[:, jc0:jc0 + 2, :], pb2[:],
                                             Act.Copy, scale=C_INIT)
                        nc.vector.tensor_copy(bp[:, jc0:jc0 + 2, :], pb2[:])
                        nc.scalar.activation(a8[:, jc0:jc0 + 2, :], pb2[:],
                                             Act.Relu, bias=nlam_a[:, 0:1],
                                             scale=C_INIT * S_A)

                # ---- phase B: 8 fp8 DoubleRow steps ----
                for step in range(NSTEPS):
                    last = step == NSTEPS - 1
                    ht8 = htp.tile([P, NDM, T], dt.float8e4, tag="ht8")
                    for dp in range(NDM // 2):
                        ph2 = psum.tile([P, 2, T], dt.float32, tag="mm")
                        for j2 in range(2):
                            dmc = dp * 2 + j2
                            for c in range(NDL // 2):
                                nc.tensor.matmul(
                                    ph2[:, j2, :],
                                    wt8[:, 2 * c:2 * c + 2,
                                        dmc * P:(dmc + 1) * P],
                                    a8[:, 2 * c:2 * c + 2, :],
                                    start=(c == 0),
                                    stop=(c == NDL // 2 - 1),
                                    perf_mode=DR)
                        nc.scalar.activation(ht8[:, 2 * dp:2 * dp + 2, :],
                                             ph2[:], Act.Copy,
                                             scale=S_H / (S_W * S_A))
                    for jp in range(NDL // 2):
                        py2 = psum.tile([P, 2, T], dt.float32, tag="mm")
                        for j2 in range(2):
                            jc = 2 * jp + j2
                            for ki, kp in enumerate((0, 1, 2, 3)):
                                nc.tensor.matmul(
                                    py2[:, j2, :],
                                    w018[:, 2 * kp:2 * kp + 2,
                                         jc * P:(jc + 1) * P],
                                    ht8[:, 2 * kp:2 * kp + 2, :],
                                    start=(ki == 0), stop=False,
                                    perf_mode=DR)
                            # -g2*a correction via fp8 diag stationary
                            nc.tensor.matmul(
                                py2[:, j2, :], dd8[:, 2 * jc:2 * jc + 2, :],
                                a8[:, jc:jc + 2, :],
                                start=False, stop=True, perf_mode=DR)
                        jc0 = 2 * jp
                        # u' = 0.9u + bp - inv*py, split as
                        #   q = -inv*py + bp   (DVE: only engine with psum+stt)
                        #   v = 0.9u           (alternate ACT/DVE)
                        #   u' = v + q         (POOL pure tensor add)
                        q2 = tmp.tile([P, 2, T], dt.bfloat16, tag="q")
                        nc.vector.scalar_tensor_tensor(
                            q2[:], py2[:], -INV_SY, bp[:, jc0:jc0 + 2, :],
                            op0=Alu.mult, op1=Alu.add)
                        v2 = tmp.tile([P, 2, T], dt.bfloat16, tag="v")
                        if jp % 4 in V_ACT:
                            nc.scalar.activation(v2[:], u[:, jc0:jc0 + 2, :],
                                                 Act.Copy, scale=0.9)
                        else:
                            nc.vector.tensor_scalar_mul(
                                v2[:], u[:, jc0:jc0 + 2, :], 0.9)
                        # final add: mostly POOL (only engine with slack),
                        # every 4th on DVE to keep POOL under the PE time
                        eng = nc.vector if jp % 4 in TT_DVE else nc.gpsimd
                        eng.tensor_tensor(u[:, jc0:jc0 + 2, :],
                                          v2[:], q2[:], op=Alu.add)
                        if jp >= NDL // 2 - 2:
                            grps = [(jc0, 2)]          # tail: pairs
                        elif jp % 2 == 1:
                            grps = [(jc0 - 2, 4)]      # quad [jq, jq+4)
                        else:
                            grps = []
                        for jq, glen in grps:
                            if not last:
                                nc.scalar.activation(a8[:, jq:jq + glen, :],
                                                     u[:, jq:jq + glen, :],
                                                     Act.Relu,
                                                     bias=nlam_a[:, 0:1],
                                                     scale=S_A)
                            else:
                                # bf16 activations for the final matmul,
                                # stored into bp's buffer (bp is dead now)
                                nc.scalar.activation(bp[:, jq:jq + glen, :],
                                                     u[:, jq:jq + glen, :],
                                                     Act.Relu,
                                                     bias=nlam[:, 0:1])

                # ---- phase C: out = a @ W^T (bf16), [tok, dm] layout ----
                for nh in range(DM // T):
                    pcs = [psum.tile([P, 2, T], dt.float32, tag="mm",
                                     name=f"pc{s}_{blk}_{nh}")
                           for s in range((T // P) // 2)]
                    for piece in range(4):
                        wsc = wstr.tile([P, NDM, T], dt.bfloat16, tag="ws")
                        nc.sync.dma_start(
                            wsc[:], wtbf_d[:, piece * 8:piece * 8 + 8,
                                           nh * T:(nh + 1) * T])
                        for kl in range(8):
                            kc = piece * 8 + kl
                            for sub in range(T // P):
                                nc.tensor.matmul(
                                    pcs[sub // 2][:, sub % 2, :],
                                    bp[:, kc, sub * P:(sub + 1) * P],
                                    wsc[:, kl, :],
                                    start=(kc == 0), stop=(kc == NDL - 1))
                    for sp in range((T // P) // 2):
                        ob2 = oio.tile([P, 2, T], dt.float32, tag="ob")
                        nc.scalar.activation(ob2[:], pcs[sp][:], Act.Copy)
                        for j2 in range(2):
                            row = blk * T + (sp * 2 + j2) * P
                            nc.sync.dma_start(
                                out_d[row:row + P, nh * T:(nh + 1) * T],
                                ob2[:, j2, :])

    nc.compile()
    return nc


def _get_nc():
    if "nc" not in _CACHE:
        _CACHE["nc"] = _build_nc()
    return _CACHE["nc"]


def _prep_shared(W):
    W = np.asarray(W, np.float32)
    wt_l = np.ascontiguousarray(
        W.T.reshape(NDL, P, DM).transpose(1, 0, 2))           # [P, NDL, DM]
    w01_l = np.ascontiguousarray(
        (0.1 * W).reshape(NDM, P, DL).transpose(1, 0, 2))     # [P, NDM, DL]
    wt8 = (wt_l * S_W).astype(F8)
    w018 = (w01_l * S_W01).astype(F8)
    w01bf = w01_l.astype(BF16)
    wtbf = wt_l.astype(BF16)
    g2 = 0.1 * (W.astype(np.float64) ** 2).sum(0).astype(np.float32)
    dval = -(g2 * (S_W01 * S_H / S_A))                        # [DL]
    dd8 = np.zeros((P, 2 * NDL, P), np.float32)
    for j in range(NDL):
        np.fill_diagonal(dd8[:, 2 * j, :], dval[j * P:(j + 1) * P])
    dd8 = dd8.astype(F8)
    return wt8, w018, dd8, w01bf, wtbf


def make_in_maps(x, W):
    x = np.asarray(x)
    xf = x.reshape(-1, DM).astype(np.float32)
    wt8, w018, dd8, w01bf, wtbf = _prep_shared(W)

    in_maps = []
    for c in range(NCORES):
        xs = xf[c * TOK_CORE:(c + 1) * TOK_CORE]
        xt = np.ascontiguousarray(
            xs.reshape(NBLK, T, NDM, P).transpose(0, 3, 2, 1)).astype(BF16)
        in_maps.append({"xt": xt, "wt8": wt8, "w018": w018, "dd8": dd8,
                        "w01bf": w01bf, "wtbf": wtbf})
    return in_maps


def kernel(x, W):
    import os

    from concourse.bass_utils import run_bass_kernel_spmd

    if not TRACE:
        os.environ.setdefault("BASS_NEVER_TRACE", "1")
    x = np.asarray(x)
    orig_shape = x.shape
    in_maps = make_in_maps(x, W)

    nc = _get_nc()
    res = run_bass_kernel_spmd(nc, in_maps, core_ids=list(range(NCORES)),
                               trace=TRACE)
    global LAST_RESULT
    LAST_RESULT = res
    out = np.concatenate([res.results[c]["out"] for c in range(NCORES)], axis=0)
    return out.reshape(orig_shape).astype(np.float32)


# revision 33
# speedup vs baseline: 1.2729x; 1.2729x over previous
"""LCA layer kernel for Trainium2, data-parallel over tokens on 8 NeuronCores.

Reference computation (per token row x of d_model=1024, W [1024, 4096]):
    b = x @ W;  G = W^T W with zero diag;  u_0 = 0
    10x: a = relu(u - 0.1); u = 0.9 u + 0.1 (b - a @ G)
    out = relu(u - 0.1) @ W^T

Device algorithm (per core, 1024 tokens = 2 blocks of T=512, all in SBUF):
  * factor a@G = (a @ W^T) @ W - g * a  with g = diag(W^T W); fold dt/tau
    into W01 = 0.1 W so B' = 0.1 b and the update is
        u' = 0.9 u + B' - (a W^T) W01 + g2*a,   g2 = 0.1 g.
  * u1 = B' exactly (u0=0); a1/a2 are ~94/80% zero, so their G-terms are
    dropped: u3 = 2.71 B' free init, then 7 iterated steps instead of 9
    (numerically validated, ~1e-3 of the error budget).
  * the 7 iterated steps run entirely in fp8e4 with DoubleRow matmuls
    (2 fp8 MACs/PE/cycle, 0.5 cyc/row): a8 = fp8(s_a relu(u-.1)),
    ht8 = fp8(s_h a W^T); the y-psum accumulates w018 x ht8 PLUS a
    diagonal fp8 stationary pair that applies the +g2*a correction
    inside the same matmul group (no elementwise op needed for it).
  * u, B' stored bf16; per 2-chunk pair the update is spread over three
    engines (scalar_tensor_tensor runs 1x on DVE only, tensor_scalar
    gets the 4x mode, plain tensor adds are legal on GPSIMD):
        q = bf16(-py/(s_w01 s_h) + bp)   DVE stt (only psum-capable)
        v = bf16(0.9 u)                  DVE ts-mul / ACT copy (by parity)
        u' = v + q                       GPSIMD / DVE tensor add
        a8 = fp8(relu(s_a u' - .1 s_a))  ACT, in quads; pairs at the
                                         step tail to unblock next step
  * first (B' = x@W01) and last (out = a@W^T) matmuls stay bf16 for
    accuracy; their bf16 weights are streamed from HBM in 16KB/partition
    pieces and never kept resident (SBUF holds fp8 weights + state).
  Emulated end-to-end error vs fp32 reference: rel_l2 ~ 1.06e-2
  (measured on hardware: 1.02e-2; gate is 2e-2).
"""

import numpy as np
import ml_dtypes

P = 128          # partitions
T = 512          # tokens per block
NBLK = 2         # blocks per core (2*512 = 1024 tokens/core)
NSTEPS = 7       # iterated fp8 steps (u4..u10); u3 = 2.71 B' is free
C_INIT = 1.0 + 0.9 + 0.81   # u3 = 2.71 B' (a1@G, a2@G dropped -- validated)
DM = 1024        # d_model
DL = 4096        # d_lca
NDM = DM // P    # 8 d_model chunks
NDL = DL // P    # 32 d_lca chunks
NCORES = 8
TOK_CORE = NBLK * T

S_W = 1024.0     # fp8 scale for W^T (ht matmul stationary)
S_W01 = 4096.0   # fp8 scale for 0.1*W (y matmul stationary)
S_A = 16.0       # fp8 scale for a
S_H = 8.0        # fp8 scale for ht
INV_SY = 1.0 / (S_W01 * S_H)

BF16 = ml_dtypes.bfloat16
F8 = ml_dtypes.float8_e4m3    # TRN fp8e4 (max +-240)

_CACHE = {}

TRACE = False
V_ACT = (1, 2)  # jp%4 values whose v-op runs on ACT
TT_DVE = (1, 3)  # jp%4 values whose final add runs on DVE
LAST_RESULT = None


def apply_reduced():
    """Shrink work (same tensor ranks/layouts) for overhead calibration."""
    global NSTEPS, NBLK, TOK_CORE
    NSTEPS = 1
    NBLK = 1
    TOK_CORE = NBLK * T


def _build_nc():
    import concourse.bacc as bacc
    import concourse.tile as tile
    import concourse.mybir as mybir

    dt = mybir.dt
    Alu = mybir.AluOpType
    Act = mybir.ActivationFunctionType
    DR = mybir.MatmulPerfMode.DoubleRow

    nc = bacc.Bacc("TRN2", target_bir_lowering=False, debug=False,
                   num_devices=NCORES)

    xt_d = nc.dram_tensor("xt", [NBLK, P, NDM, T], dt.bfloat16,
                          kind="ExternalInput").ap()
    wt8_d = nc.dram_tensor("wt8", [P, NDL, DM], dt.float8e4,
                           kind="ExternalInput").ap()
    w018_d = nc.dram_tensor("w018", [P, NDM, DL], dt.float8e4,
                            kind="ExternalInput").ap()
    dd8_d = nc.dram_tensor("dd8", [P, 2 * NDL, P], dt.float8e4,
                           kind="ExternalInput").ap()
    w01bf_d = nc.dram_tensor("w01bf", [P, NDM, DL], dt.bfloat16,
                             kind="ExternalInput").ap()
    wtbf_d = nc.dram_tensor("wtbf", [P, NDL, DM], dt.bfloat16,
                            kind="ExternalInput").ap()
    out_d = nc.dram_tensor("out", [TOK_CORE, DM], dt.float32,
                           kind="ExternalOutput").ap()

    with tile.TileContext(nc) as tc:
        with (
            tc.tile_pool(name="wpool", bufs=1) as wpool,
            tc.tile_pool(name="state", bufs=1) as state,
            tc.tile_pool(name="htp", bufs=2) as htp,
            tc.tile_pool(name="xio", bufs=1) as xio,
            tc.tile_pool(name="wstr", bufs=2) as wstr,
            tc.tile_pool(name="oio", bufs=1) as oio,
            tc.tile_pool(name="tmp", bufs=4) as tmp,
            tc.tile_pool(name="psum", bufs=4, space="PSUM") as psum,
        ):
            # ---- resident fp8 weights ----
            wt8 = wpool.tile([P, NDL, DM], dt.float8e4, tag="wt8")
            w018 = wpool.tile([P, NDM, DL], dt.float8e4, tag="w018")
            dd8 = wpool.tile([P, 2 * NDL, P], dt.float8e4, tag="dd8")
            nlam_a = wpool.tile([P, 1], dt.float32, tag="nlam_a")
            nlam = wpool.tile([P, 1], dt.float32, tag="nlam")
            nc.gpsimd.memset(nlam_a[:], -0.1 * S_A)
            nc.gpsimd.memset(nlam[:], -0.1)

            first_dma_issued = False

            for blk in range(NBLK):
                xt = xio.tile([P, NDM, T], dt.bfloat16, tag="xt")
                nc.sync.dma_start(xt[:], xt_d[blk])

                u = state.tile([P, NDL, T], dt.bfloat16, tag="u")
                bp = state.tile([P, NDL, T], dt.bfloat16, tag="bp")
                a8 = state.tile([P, NDL + 1, T], dt.float8e4, tag="a8")
                if blk == 0:
                    nc.gpsimd.memset(a8[:, NDL, :], 0.0)

                # ---- phase A: B' = x @ W01 (bf16), u=1.9B', bp=B' ----
                for jhg in range(DL // T):
                    ws = wstr.tile([P, NDM, T], dt.bfloat16, tag="ws")
                    nc.sync.dma_start(
                        ws[:], w01bf_d[:, :, jhg * T:(jhg + 1) * T])
                    if not first_dma_issued:
                        # resident weights after the first A-piece so the
                        # pipeline head isn't DMA-starved
                        first_dma_issued = True
                        for kg in range(0, NDL, 8):
                            nc.sync.dma_start(wt8[:, kg:kg + 8, :],
                                              wt8_d[:, kg:kg + 8, :])
                        for mg in range(0, NDM, 2):
                            nc.sync.dma_start(w018[:, mg:mg + 2, :],
                                              w018_d[:, mg:mg + 2, :])
                        nc.sync.dma_start(dd8[:], dd8_d[:])
                    for jp in range(T // (2 * P)):
                        jc0 = jhg * (T // P) + jp * 2
                        pb2 = psum.tile([P, 2, T], dt.float32, tag="mm")
                        for j2 in range(2):
                            jl = jp * 2 + j2
                            for dmc in range(NDM):
                                nc.tensor.matmul(
                                    pb2[:, j2, :],
                                    ws[:, dmc, jl * P:(jl + 1) * P],
                                    xt[:, dmc, :],
                                    start=(dmc == 0), stop=(dmc == NDM - 1))
                        nc.scalar.activation(u[:, jc0:jc0 + 2, :], pb2[:],
                                             Act.Copy, scale=C_INIT)
                        nc.vector.tensor_copy(bp[:, jc0:jc0 + 2, :], pb2[:])
                        nc.scalar.activation(a8[:, jc0:jc0 + 2, :], pb2[:],
                                             Act.Relu, bias=nlam_a[:, 0:1],
                                             scale=C_INIT * S_A)

                # ---- phase B: 8 fp8 DoubleRow steps ----
                for step in range(NSTEPS):
                    last = step == NSTEPS - 1
                    ht8 = htp.tile([P, NDM, T], dt.float8e4, tag="ht8")
                    for dp in range(NDM // 2):
                        ph2 = psum.tile([P, 2, T], dt.float32, tag="mm")
                        for j2 in range(2):
                            dmc = dp * 2 + j2
                            for c in range(NDL // 2):
                                nc.tensor.matmul(
                                    ph2[:, j2, :],
                                    wt8[:, 2 * c:2 * c + 2,
                                        dmc * P:(dmc + 1) * P],
                                    a8[:, 2 * c:2 * c + 2, :],
                                    start=(c == 0),
                                    stop=(c == NDL // 2 - 1),
                                    perf_mode=DR)
                        nc.scalar.activation(ht8[:, 2 * dp:2 * dp + 2, :],
                                             ph2[:], Act.Copy,
                                             scale=S_H / (S_W * S_A))
                    for jp in range(NDL // 2):
                        py2 = psum.tile([P, 2, T], dt.float32, tag="mm")
                        for j2 in range(2):
                            jc = 2 * jp + j2
                            for ki, kp in enumerate((0, 1, 2, 3)):
                                nc.tensor.matmul(
                                    py2[:, j2, :],
                                    w018[:, 2 * kp:2 * kp + 2,
                                         jc * P:(jc + 1) * P],
                                    ht8[:, 2 * kp:2 * kp + 2, :],
                                    start=(ki == 0), stop=False,
                                    perf_mode=DR)
                            # -g2*a correction via fp8 diag stationary
                            nc.tensor.matmul(
                                py2[:, j2, :], dd8[:, 2 * jc:2 * jc + 2, :],
                                a8[:, jc:jc + 2, :],
                                start=False, stop=True, perf_mode=DR)
                        jc0 = 2 * jp
                        # u' = 0.9u + bp - inv*py, split as
                        #   q = -inv*py + bp   (DVE: only engine with psum+stt)
                        #   v = 0.9u           (alternate ACT/DVE)
                        #   u' = v + q         (POOL pure tensor add)
                        q2 = tmp.tile([P, 2, T], dt.bfloat16, tag="q")
                        nc.vector.scalar_tensor_tensor(
                            q2[:], py2[:], -INV_SY, bp[:, jc0:jc0 + 2, :],
                            op0=Alu.mult, op1=Alu.add)
                        v2 = tmp.tile([P, 2, T], dt.bfloat16, tag="v")
                        if jp % 4 in V_ACT:
                            nc.scalar.activation(v2[:], u[:, jc0:jc0 + 2, :],
                                                 Act.Copy, scale=0.9)
                        else:
                            nc.vector.tensor_scalar_mul(
                                v2[:], u[:, jc0:jc0 + 2, :], 0.9)
                        # final add: mostly POOL (only engine with slack),
                        # every 4th on DVE to keep POOL under the PE time
                        eng = nc.vector if jp % 4 in TT_DVE else nc.gpsimd
                        eng.tensor_tensor(u[:, jc0:jc0 + 2, :],
                                          v2[:], q2[:], op=Alu.add)
                        if jp >= NDL // 2 - 2:
                            grps = [(jc0, 2)]          # tail: pairs
                        elif jp % 2 == 1:
                            grps = [(jc0 - 2, 4)]      # quad [jq, jq+4)
                        else:
                            grps = []
                        for jq, glen in grps:
                            if not last:
                                nc.scalar.activation(a8[:, jq:jq + glen, :],
                                                     u[:, jq:jq + glen, :],
                                                     Act.Relu,
                                                     bias=nlam_a[:, 0:1],
                                                     scale=S_A)
                            else:
                                # bf16 activations for the final matmul,
                                # stored into bp's buffer (bp is dead now)
                                nc.scalar.activation(bp[:, jq:jq + glen, :],
                                                     u[:, jq:jq + glen, :],
                                                     Act.Relu,
                                                     bias=nlam[:, 0:1])

                # ---- phase C: out = a @ W^T (bf16), [tok, dm] layout ----
                for nh in range(DM // T):
                    pcs = [psum.tile([P, 2, T], dt.float32, tag="mm",
                                     name=f"pc{s}_{blk}_{nh}")
                           for s in range((T // P) // 2)]
                    for piece in range(4):
                        wsc = wstr.tile([P, NDM, T], dt.bfloat16, tag="ws")
                        nc.sync.dma_start(
                            wsc[:], wtbf_d[:, piece * 8:piece * 8 + 8,
                                           nh * T:(nh + 1) * T])
                        for kl in range(8):
                            kc = piece * 8 + kl
                            for sub in range(T // P):
                                nc.tensor.matmul(
                                    pcs[sub // 2][:, sub % 2, :],
                                    bp[:, kc, sub * P:(sub + 1) * P],
                                    wsc[:, kl, :],
                                    start=(kc == 0), stop=(kc == NDL - 1))
                    for sp in range((T // P) // 2):
                        ob2 = oio.tile([P, 2, T], dt.float32, tag="ob")
                        nc.scalar.activation(ob2[:], pcs[sp][:], Act.Copy)
                        for j2 in range(2):
                            row = blk * T + (sp * 2 + j2) * P
                            nc.sync.dma_start(
                                out_d[row:row + P, nh * T:(nh + 1) * T],
                                ob2[:, j2, :])

    nc.compile()
    return nc


def _get_nc():
    if "nc" not in _CACHE:
        _CACHE["nc"] = _build_nc()
    return _CACHE["nc"]


def _prep_shared(W):
    W = np.asarray(W, np.float32)
    wt_l = np.ascontiguousarray(
        W.T.reshape(NDL, P, DM).transpose(1, 0, 2))           # [P, NDL, DM]
    w01_l = np.ascontiguousarray(
        (0.1 * W).reshape(NDM, P, DL).transpose(1, 0, 2))     # [P, NDM, DL]
    wt8 = (wt_l * S_W).astype(F8)
    w018 = (w01_l * S_W01).astype(F8)
    w01bf = w01_l.astype(BF16)
    wtbf = wt_l.astype(BF16)
    g2 = 0.1 * (W.astype(np.float64) ** 2).sum(0).astype(np.float32)
    dval = -(g2 * (S_W01 * S_H / S_A))                        # [DL]
    dd8 = np.zeros((P, 2 * NDL, P), np.float32)
    for j in range(NDL):
        np.fill_diagonal(dd8[:, 2 * j, :], dval[j * P:(j + 1) * P])
    dd8 = dd8.astype(F8)
    return wt8, w018, dd8, w01bf, wtbf


def make_in_maps(x, W):
    x = np.asarray(x)
    xf = x.reshape(-1, DM).astype(np.float32)
    wt8, w018, dd8, w01bf, wtbf = _prep_shared(W)

    in_maps = []
    for c in range(NCORES):
        xs = xf[c * TOK_CORE:(c + 1) * TOK_CORE]
        xt = np.ascontiguousarray(
            xs.reshape(NBLK, T, NDM, P).transpose(0, 3, 2, 1)).astype(BF16)
        in_maps.append({"xt": xt, "wt8": wt8, "w018": w018, "dd8": dd8,
                        "w01bf": w01bf, "wtbf": wtbf})
    return in_maps


def kernel(x, W):
    import os

    from concourse.bass_utils import run_bass_kernel_spmd

    if not TRACE:
        os.environ.setdefault("BASS_NEVER_TRACE", "1")
    x = np.asarray(x)
    orig_shape = x.shape
    in_maps = make_in_maps(x, W)

    nc = _get_nc()
    res = run_bass_kernel_spmd(nc, in_maps, core_ids=list(range(NCORES)),
                               trace=TRACE)
    global LAST_RESULT
    LAST_RESULT = res
    out = np.concatenate([res.results[c]["out"] for c in range(NCORES)], axis=0)
    return out.reshape(orig_shape).astype(np.float32)


# revision 35
# speedup vs baseline: 1.5023x; 1.1802x over previous
"""LCA layer kernel for Trainium2, data-parallel over tokens on 8 NeuronCores.

Reference computation (per token row x of d_model=1024, W [1024, 4096]):
    b = x @ W;  G = W^T W with zero diag;  u_0 = 0
    10x: a = relu(u - 0.1); u = 0.9 u + 0.1 (b - a @ G)
    out = relu(u - 0.1) @ W^T

Device algorithm (per core, 1024 tokens = 2 blocks of T=512, all in SBUF):
  * factor a@G = (a @ W^T) @ W - g * a  with g = diag(W^T W); fold dt/tau
    into W01 = 0.1 W so B' = 0.1 b and the update is
        u' = 0.9 u + B' - (a W^T) W01 + g2*a,   g2 = 0.1 g.
  * u1 = B' exactly (u0=0); a1/a2 are ~94/80% zero, so their G-terms are
    dropped: u3 = 2.71 B' free init, then 7 iterated steps instead of 9
    (numerically validated, ~1e-3 of the error budget).
  * the 7 iterated steps run entirely in fp8e4 with DoubleRow matmuls
    (2 fp8 MACs/PE/cycle, 0.5 cyc/row): a8 = fp8(s_a relu(u-.1)),
    ht8 = fp8(s_h a W^T); the y-psum accumulates w018 x ht8 PLUS a
    diagonal fp8 stationary pair that applies the +g2*a correction
    inside the same matmul group (no elementwise op needed for it).
  * u, B' stored bf16; per 2-chunk pair the update is spread over three
    engines (scalar_tensor_tensor runs 1x on DVE only, tensor_scalar
    gets the 4x mode, plain tensor adds are legal on GPSIMD):
        q = bf16(-py/(s_w01 s_h) + bp)   DVE stt (only psum-capable)
        v = bf16(0.9 u)                  DVE ts-mul / ACT copy (by parity)
        u' = v + q                       GPSIMD / DVE tensor add
        a8 = fp8(relu(s_a u' - .1 s_a))  ACT, in quads; pairs at the
                                         step tail to unblock next step
  * first (B' = x@W01) and last (out = a@W^T) matmuls stay bf16 for
    accuracy; their bf16 weights are streamed from HBM in 16KB/partition
    pieces and never kept resident (SBUF holds fp8 weights + state).
  Emulated end-to-end error vs fp32 reference: rel_l2 ~ 1.06e-2
  (measured on hardware: 1.02e-2; gate is 2e-2).
"""

import numpy as np
import ml_dtypes

P = 128          # partitions
T = 512          # tokens per block
NBLK = 2         # blocks per core (2*512 = 1024 tokens/core)
NSTEPS = 7       # iterated fp8 steps (u4..u10); u3 = 2.71 B' is free
C_INIT = 1.0 + 0.9 + 0.81   # u3 = 2.71 B' (a1@G, a2@G dropped -- validated)
DM = 1024        # d_model
DL = 4096        # d_lca
NDM = DM // P    # 8 d_model chunks
NDL = DL // P    # 32 d_lca chunks
NCORES = 8
TOK_CORE = NBLK * T

S_W = 1024.0     # fp8 scale for W^T (ht matmul stationary)
S_W01 = 4096.0   # fp8 scale for 0.1*W (y matmul stationary)
S_A = 16.0       # fp8 scale for a
S_H = 8.0        # fp8 scale for ht
INV_SY = 1.0 / (S_W01 * S_H)

BF16 = ml_dtypes.bfloat16
F8 = ml_dtypes.float8_e4m3    # TRN fp8e4 (max +-240)

_CACHE = {}

TRACE = False
V_ACT = (0, 2)  # jp%4 values whose v-op runs on ACT
TT_DVE = (1, 3)  # jp%4 values whose final add runs on DVE
LAST_RESULT = None


def apply_reduced():
    """Shrink work (same tensor ranks/layouts) for overhead calibration."""
    global NSTEPS, NBLK, TOK_CORE
    NSTEPS = 1
    NBLK = 1
    TOK_CORE = NBLK * T


def _build_nc():
    import concourse.bacc as bacc
    import concourse.tile as tile
    import concourse.mybir as mybir

    dt = mybir.dt
    Alu = mybir.AluOpType
    Act = mybir.ActivationFunctionType
    DR = mybir.MatmulPerfMode.DoubleRow

    nc = bacc.Bacc("TRN2", target_bir_lowering=False, debug=False,
                   num_devices=NCORES)

    xt_d = nc.dram_tensor("xt", [NBLK, P, NDM, T], dt.bfloat16,
                          kind="ExternalInput").ap()
    wt8_d = nc.dram_tensor("wt8", [P, NDL, DM], dt.float8e4,
                           kind="ExternalInput").ap()
    w018_d = nc.dram_tensor("w018", [P, NDM, DL], dt.float8e4,
                            kind="ExternalInput").ap()
    dd8_d = nc.dram_tensor("dd8", [P, 2 * NDL, P], dt.float8e4,
                           kind="ExternalInput").ap()
    w01bf_d = nc.dram_tensor("w01bf", [P, NDM, DL], dt.bfloat16,
                             kind="ExternalInput").ap()
    wtbf_d = nc.dram_tensor("wtbf", [P, NDL, DM], dt.bfloat16,
                            kind="ExternalInput").ap()
    out_d = nc.dram_tensor("out", [TOK_CORE, DM], dt.float32,
                           kind="ExternalOutput").ap()

    with tile.TileContext(nc) as tc:
        with (
            tc.tile_pool(name="wpool", bufs=1) as wpool,
            tc.tile_pool(name="state", bufs=1) as state,
            tc.tile_pool(name="htp", bufs=2) as htp,
            tc.tile_pool(name="xio", bufs=1) as xio,
            tc.tile_pool(name="wstr", bufs=2) as wstr,
            tc.tile_pool(name="oio", bufs=1) as oio,
            tc.tile_pool(name="tmp", bufs=4) as tmp,
            tc.tile_pool(name="psum", bufs=4, space="PSUM") as psum,
        ):
            # ---- resident fp8 weights ----
            wt8 = wpool.tile([P, NDL, DM], dt.float8e4, tag="wt8")
            w018 = wpool.tile([P, NDM, DL], dt.float8e4, tag="w018")
            dd8 = wpool.tile([P, 2 * NDL, P], dt.float8e4, tag="dd8")
            nlam_a = wpool.tile([P, 1], dt.float32, tag="nlam_a")
            nlam = wpool.tile([P, 1], dt.float32, tag="nlam")
            nc.gpsimd.memset(nlam_a[:], -0.1 * S_A)
            nc.gpsimd.memset(nlam[:], -0.1)

            first_dma_issued = False

            for blk in range(NBLK):
                xt = xio.tile([P, NDM, T], dt.bfloat16, tag="xt")
                nc.sync.dma_start(xt[:], xt_d[blk])

                u = state.tile([P, NDL, T], dt.bfloat16, tag="u")
                bp = state.tile([P, NDL, T], dt.bfloat16, tag="bp")
                a8 = state.tile([P, NDL + 1, T], dt.float8e4, tag="a8")
                if blk == 0:
                    nc.gpsimd.memset(a8[:, NDL, :], 0.0)

                # ---- phase A: B' = x @ W01 (bf16), u=1.9B', bp=B' ----
                for jhg in range(DL // T):
                    ws = wstr.tile([P, NDM, T], dt.bfloat16, tag="ws")
                    nc.sync.dma_start(
                        ws[:], w01bf_d[:, :, jhg * T:(jhg + 1) * T])
                    if not first_dma_issued:
                        # resident weights after the first A-piece so the
                        # pipeline head isn't DMA-starved
                        first_dma_issued = True
                        for kg in range(0, NDL, 8):
                            nc.sync.dma_start(wt8[:, kg:kg + 8, :],
                                              wt8_d[:, kg:kg + 8, :])
                        for mg in range(0, NDM, 2):
                            nc.sync.dma_start(w018[:, mg:mg + 2, :],
                                              w018_d[:, mg:mg + 2, :])
                        nc.sync.dma_start(dd8[:], dd8_d[:])
                    for jp in range(T // (2 * P)):
                        jc0 = jhg * (T // P) + jp * 2
                        pb2 = psum.tile([P, 2, T], dt.float32, tag="mm")
                        for j2 in range(2):
                            jl = jp * 2 + j2
                            for dmc in range(NDM):
                                nc.tensor.matmul(
                                    pb2[:, j2, :],
                                    ws[:, dmc, jl * P:(jl + 1) * P],
                                    xt[:, dmc, :],
                                    start=(dmc == 0), stop=(dmc == NDM - 1))
                        nc.scalar.activation(u[:, jc0:jc0 + 2, :], pb2[:],
                                             Act.Copy, scale=C_INIT)
                        nc.vector.tensor_copy(bp[:, jc0:jc0 + 2, :], pb2[:])
                        nc.scalar.activation(a8[:, jc0:jc0 + 2, :], pb2[:],
                                             Act.Relu, bias=nlam_a[:, 0:1],
                                             scale=C_INIT * S_A)

                # ---- phase B: 8 fp8 DoubleRow steps ----
                for step in range(NSTEPS):
                    last = step == NSTEPS - 1
                    ht8 = htp.tile([P, NDM, T], dt.float8e4, tag="ht8")
                    for dp in range(NDM // 2):
                        ph2 = psum.tile([P, 2, T], dt.float32, tag="mm")
                        for j2 in range(2):
                            dmc = dp * 2 + j2
                            for c in range(NDL // 2):
                                nc.tensor.matmul(
                                    ph2[:, j2, :],
                                    wt8[:, 2 * c:2 * c + 2,
                                        dmc * P:(dmc + 1) * P],
                                    a8[:, 2 * c:2 * c + 2, :],
                                    start=(c == 0),
                                    stop=(c == NDL // 2 - 1),
                                    perf_mode=DR)
                        nc.scalar.activation(ht8[:, 2 * dp:2 * dp + 2, :],
                                             ph2[:], Act.Copy,
                                             scale=S_H / (S_W * S_A))
                    # u' = 0.9u + bp - inv*py.  t = 0.9u + bp is
                    # precomputed 2-3 pairs AHEAD of the matmuls (v on
                    # DVE 4x-mode / ACT, add on POOL/DVE), so after each
                    # psum lands only one DVE stt + the ACT relu remain
                    # on the critical path.
                    tts = {}

                    def emit_vt(jp):
                        jc0 = 2 * jp
                        v2 = tmp.tile([P, 2, T], dt.bfloat16, tag="v",
                                      name=f"v_{jp}")
                        if jp % 4 in V_ACT:
                            nc.scalar.activation(v2[:], u[:, jc0:jc0 + 2, :],
                                                 Act.Copy, scale=0.9)
                        else:
                            nc.vector.tensor_scalar_mul(
                                v2[:], u[:, jc0:jc0 + 2, :], 0.9)
                        t2 = tmp.tile([P, 2, T], dt.bfloat16, tag="q",
                                      name=f"t_{jp}")
                        eng = nc.vector if jp % 4 in TT_DVE else nc.gpsimd
                        eng.tensor_tensor(t2[:], v2[:],
                                          bp[:, jc0:jc0 + 2, :], op=Alu.add)
                        tts[jp] = t2

                    for jp in range(NDL // 2):
                        if jp == 0:
                            emit_vt(0)
                            emit_vt(1)
                        if jp + 2 < NDL // 2:
                            emit_vt(jp + 2)
                        py2 = psum.tile([P, 2, T], dt.float32, tag="mm")
                        for j2 in range(2):
                            jc = 2 * jp + j2
                            for ki, kp in enumerate((0, 1, 2, 3)):
                                nc.tensor.matmul(
                                    py2[:, j2, :],
                                    w018[:, 2 * kp:2 * kp + 2,
                                         jc * P:(jc + 1) * P],
                                    ht8[:, 2 * kp:2 * kp + 2, :],
                                    start=(ki == 0), stop=False,
                                    perf_mode=DR)
                            # -g2*a correction via fp8 diag stationary
                            nc.tensor.matmul(
                                py2[:, j2, :], dd8[:, 2 * jc:2 * jc + 2, :],
                                a8[:, jc:jc + 2, :],
                                start=False, stop=True, perf_mode=DR)
                        jc0 = 2 * jp
                        nc.vector.scalar_tensor_tensor(
                            u[:, jc0:jc0 + 2, :], py2[:], -INV_SY,
                            tts.pop(jp)[:], op0=Alu.mult, op1=Alu.add)
                        if jp >= NDL // 2 - 2:
                            grps = [(jc0, 2)]          # tail: pairs
                        elif jp % 2 == 1:
                            grps = [(jc0 - 2, 4)]      # quad [jq, jq+4)
                        else:
                            grps = []
                        for jq, glen in grps:
                            if not last:
                                nc.scalar.activation(a8[:, jq:jq + glen, :],
                                                     u[:, jq:jq + glen, :],
                                                     Act.Relu,
                                                     bias=nlam_a[:, 0:1],
                                                     scale=S_A)
                            else:
                                # bf16 activations for the final matmul,
                                # stored into bp's buffer (bp is dead now)
                                nc.scalar.activation(bp[:, jq:jq + glen, :],
                                                     u[:, jq:jq + glen, :],
                                                     Act.Relu,
                                                     bias=nlam[:, 0:1])

                # ---- phase C: out = a @ W^T (bf16), [tok, dm] layout ----
                for nh in range(DM // T):
                    pcs = [psum.tile([P, 2, T], dt.float32, tag="mm",
                                     name=f"pc{s}_{blk}_{nh}")
                           for s in range((T // P) // 2)]
                    for piece in range(4):
                        wsc = wstr.tile([P, NDM, T], dt.bfloat16, tag="ws")
                        nc.sync.dma_start(
                            wsc[:], wtbf_d[:, piece * 8:piece * 8 + 8,
                                           nh * T:(nh + 1) * T])
                        for kl in range(8):
                            kc = piece * 8 + kl
                            for sub in range(T // P):
                                nc.tensor.matmul(
                                    pcs[sub // 2][:, sub % 2, :],
                                    bp[:, kc, sub * P:(sub + 1) * P],
                                    wsc[:, kl, :],
                                    start=(kc == 0), stop=(kc == NDL - 1))
                    for sp in range((T // P) // 2):
                        ob2 = oio.tile([P, 2, T], dt.float32, tag="ob")
                        nc.scalar.activation(ob2[:], pcs[sp][:], Act.Copy)
                        for j2 in range(2):
                            row = blk * T + (sp * 2 + j2) * P
                            nc.sync.dma_start(
                                out_d[row:row + P, nh * T:(nh + 1) * T],
                                ob2[:, j2, :])

    nc.compile()
    return nc


def _get_nc():
    if "nc" not in _CACHE:
        _CACHE["nc"] = _build_nc()
    return _CACHE["nc"]


def _prep_shared(W):
    W = np.asarray(W, np.float32)
    wt_l = np.ascontiguousarray(
        W.T.reshape(NDL, P, DM).transpose(1, 0, 2))           # [P, NDL, DM]
    w01_l = np.ascontiguousarray(
        (0.1 * W).reshape(NDM, P, DL).transpose(1, 0, 2))     # [P, NDM, DL]
    wt8 = (wt_l * S_W).astype(F8)
    w018 = (w01_l * S_W01).astype(F8)
    w01bf = w01_l.astype(BF16)
    wtbf = wt_l.astype(BF16)
    g2 = 0.1 * (W.astype(np.float64) ** 2).sum(0).astype(np.float32)
    dval = -(g2 * (S_W01 * S_H / S_A))                        # [DL]
    dd8 = np.zeros((P, 2 * NDL, P), np.float32)
    for j in range(NDL):
        np.fill_diagonal(dd8[:, 2 * j, :], dval[j * P:(j + 1) * P])
    dd8 = dd8.astype(F8)
    return wt8, w018, dd8, w01bf, wtbf


def make_in_maps(x, W):
    x = np.asarray(x)
    xf = x.reshape(-1, DM).astype(np.float32)
    wt8, w018, dd8, w01bf, wtbf = _prep_shared(W)

    in_maps = []
    for c in range(NCORES):
        xs = xf[c * TOK_CORE:(c + 1) * TOK_CORE]
        xt = np.ascontiguousarray(
            xs.reshape(NBLK, T, NDM, P).transpose(0, 3, 2, 1)).astype(BF16)
        in_maps.append({"xt": xt, "wt8": wt8, "w018": w018, "dd8": dd8,
                        "w01bf": w01bf, "wtbf": wtbf})
    return in_maps


def kernel(x, W):
    import os

    from concourse.bass_utils import run_bass_kernel_spmd

    if not TRACE:
        os.environ.setdefault("BASS_NEVER_TRACE", "1")
    x = np.asarray(x)
    orig_shape = x.shape
    in_maps = make_in_maps(x, W)

    nc = _get_nc()
    res = run_bass_kernel_spmd(nc, in_maps, core_ids=list(range(NCORES)),
                               trace=TRACE)
    global LAST_RESULT
    LAST_RESULT = res
    out = np.concatenate([res.results[c]["out"] for c in range(NCORES)], axis=0)
    return out.reshape(orig_shape).astype(np.float32)


# revision 37
# speedup vs baseline: 1.8259x; 1.2154x over previous
"""LCA layer kernel for Trainium2, data-parallel over tokens on 8 NeuronCores.

Reference computation (per token row x of d_model=1024, W [1024, 4096]):
    b = x @ W;  G = W^T W with zero diag;  u_0 = 0
    10x: a = relu(u - 0.1); u = 0.9 u + 0.1 (b - a @ G)
    out = relu(u - 0.1) @ W^T

Device algorithm (per core, 1024 tokens = 2 blocks of T=512, all in SBUF):
  * factor a@G = (a @ W^T) @ W - g * a  with g = diag(W^T W); fold dt/tau
    into W01 = 0.1 W so B' = 0.1 b and the update is
        u' = 0.9 u + B' - (a W^T) W01 + g2*a,   g2 = 0.1 g.
  * u1 = B' exactly (u0=0); a1/a2 are ~94/80% zero, so their G-terms are
    dropped: u3 = 2.71 B' free init, then 7 iterated steps instead of 9
    (numerically validated, ~1e-3 of the error budget).
  * the 7 iterated steps run entirely in fp8e4 with DoubleRow matmuls
    (2 fp8 MACs/PE/cycle, 0.5 cyc/row): a8 = fp8(s_a relu(u-.1)),
    ht8 = fp8(s_h a W^T); the y-psum accumulates w018 x ht8 PLUS a
    diagonal fp8 stationary pair that applies the +g2*a correction
    inside the same matmul group (no elementwise op needed for it).
  * u, B' stored bf16; per 2-chunk pair the update is spread over three
    engines (scalar_tensor_tensor runs 1x on DVE only, tensor_scalar
    gets the 4x mode, plain tensor adds are legal on GPSIMD):
        q = bf16(-py/(s_w01 s_h) + bp)   DVE stt (only psum-capable)
        v = bf16(0.9 u)                  DVE ts-mul / ACT copy (by parity)
        u' = v + q                       GPSIMD / DVE tensor add
        a8 = fp8(relu(s_a u' - .1 s_a))  ACT, in quads; pairs at the
                                         step tail to unblock next step
  * first (B' = x@W01) and last (out = a@W^T) matmuls stay bf16 for
    accuracy; their bf16 weights are streamed from HBM in 16KB/partition
    pieces and never kept resident (SBUF holds fp8 weights + state).
  Emulated end-to-end error vs fp32 reference: rel_l2 ~ 1.06e-2
  (measured on hardware: 1.02e-2; gate is 2e-2).
"""

import numpy as np
import ml_dtypes

P = 128          # partitions
T = 512          # tokens per block
NBLK = 2         # blocks per core (2*512 = 1024 tokens/core)
NSTEPS = 7       # iterated fp8 steps (u4..u10); u3 = 2.71 B' is free
C_INIT = 1.0 + 0.9 + 0.81   # u3 = 2.71 B' (a1@G, a2@G dropped -- validated)
DM = 1024        # d_model
DL = 4096        # d_lca
NDM = DM // P    # 8 d_model chunks
NDL = DL // P    # 32 d_lca chunks
NCORES = 8
TOK_CORE = NBLK * T

S_W = 1024.0     # fp8 scale for W^T (ht matmul stationary)
S_W01 = 4096.0   # fp8 scale for 0.1*W (y matmul stationary)
S_A = 16.0       # fp8 scale for a
S_H = 8.0        # fp8 scale for ht
INV_SY = 1.0 / (S_W01 * S_H)

BF16 = ml_dtypes.bfloat16
F8 = ml_dtypes.float8_e4m3    # TRN fp8e4 (max +-240)

_CACHE = {}

TRACE = False
V_ACT = (0, 2, 4, 6)   # jp%8 values whose v-op runs on ACT
TT_DVE = (1, 3, 5)     # jp%8 values whose t-add runs on DVE
LAST_RESULT = None


def apply_reduced():
    """Shrink work (same tensor ranks/layouts) for overhead calibration."""
    global NSTEPS, NBLK, TOK_CORE
    NSTEPS = 1
    NBLK = 1
    TOK_CORE = NBLK * T


def _build_nc():
    import concourse.bacc as bacc
    import concourse.tile as tile
    import concourse.mybir as mybir

    dt = mybir.dt
    Alu = mybir.AluOpType
    Act = mybir.ActivationFunctionType
    DR = mybir.MatmulPerfMode.DoubleRow

    nc = bacc.Bacc("TRN2", target_bir_lowering=False, debug=False,
                   num_devices=NCORES)

    xt_d = nc.dram_tensor("xt", [NBLK, P, NDM, T], dt.bfloat16,
                          kind="ExternalInput").ap()
    wt8_d = nc.dram_tensor("wt8", [P, NDL, DM], dt.float8e4,
                           kind="ExternalInput").ap()
    w018_d = nc.dram_tensor("w018", [P, NDM, DL], dt.float8e4,
                            kind="ExternalInput").ap()
    dd8_d = nc.dram_tensor("dd8", [P, 2 * NDL, P], dt.float8e4,
                           kind="ExternalInput").ap()
    w01bf_d = nc.dram_tensor("w01bf", [P, NDM, DL], dt.bfloat16,
                             kind="ExternalInput").ap()
    wtbf_d = nc.dram_tensor("wtbf", [P, NDL, DM], dt.bfloat16,
                            kind="ExternalInput").ap()
    out_d = nc.dram_tensor("out", [TOK_CORE, DM], dt.float32,
                           kind="ExternalOutput").ap()

    with tile.TileContext(nc) as tc:
        with (
            tc.tile_pool(name="wpool", bufs=1) as wpool,
            tc.tile_pool(name="state", bufs=1) as state,
            tc.tile_pool(name="htp", bufs=2) as htp,
            tc.tile_pool(name="xio", bufs=1) as xio,
            tc.tile_pool(name="wstr", bufs=2) as wstr,
            tc.tile_pool(name="oio", bufs=1) as oio,
            tc.tile_pool(name="tmp", bufs=4) as tmp,
            tc.tile_pool(name="psum", bufs=4, space="PSUM") as psum,
        ):
            # ---- resident fp8 weights ----
            wt8 = wpool.tile([P, NDL, DM], dt.float8e4, tag="wt8")
            w018 = wpool.tile([P, NDM, DL], dt.float8e4, tag="w018")
            dd8 = wpool.tile([P, 2 * NDL, P], dt.float8e4, tag="dd8")
            nlam_a = wpool.tile([P, 1], dt.float32, tag="nlam_a")
            nlam = wpool.tile([P, 1], dt.float32, tag="nlam")
            nc.gpsimd.memset(nlam_a[:], -0.1 * S_A)
            nc.gpsimd.memset(nlam[:], -0.1)

            first_dma_issued = False

            for blk in range(NBLK):
                xt = xio.tile([P, NDM, T], dt.bfloat16, tag="xt")
                nc.sync.dma_start(xt[:], xt_d[blk])

                u = state.tile([P, NDL, T], dt.bfloat16, tag="u")
                bp = state.tile([P, NDL, T], dt.bfloat16, tag="bp")
                a8 = state.tile([P, NDL + 1, T], dt.float8e4, tag="a8")
                if blk == 0:
                    nc.gpsimd.memset(a8[:, NDL, :], 0.0)

                # ---- phase A: B' = x @ W01 (bf16), u=1.9B', bp=B' ----
                for jhg in range(DL // T):
                    ws = wstr.tile([P, NDM, T], dt.bfloat16, tag="ws")
                    nc.sync.dma_start(
                        ws[:], w01bf_d[:, :, jhg * T:(jhg + 1) * T])
                    if not first_dma_issued:
                        # resident weights after the first A-piece so the
                        # pipeline head isn't DMA-starved
                        first_dma_issued = True
                        for kg in range(0, NDL, 8):
                            nc.sync.dma_start(wt8[:, kg:kg + 8, :],
                                              wt8_d[:, kg:kg + 8, :])
                        for mg in range(0, NDM, 2):
                            nc.sync.dma_start(w018[:, mg:mg + 2, :],
                                              w018_d[:, mg:mg + 2, :])
                        nc.sync.dma_start(dd8[:], dd8_d[:])
                    for jp in range(T // (2 * P)):
                        jc0 = jhg * (T // P) + jp * 2
                        pb2 = psum.tile([P, 2, T], dt.float32, tag="mm")
                        for j2 in range(2):
                            jl = jp * 2 + j2
                            for dmc in range(NDM):
                                nc.tensor.matmul(
                                    pb2[:, j2, :],
                                    ws[:, dmc, jl * P:(jl + 1) * P],
                                    xt[:, dmc, :],
                                    start=(dmc == 0), stop=(dmc == NDM - 1))
                        nc.scalar.activation(u[:, jc0:jc0 + 2, :], pb2[:],
                                             Act.Copy, scale=C_INIT)
                        nc.vector.tensor_copy(bp[:, jc0:jc0 + 2, :], pb2[:])
                        nc.scalar.activation(a8[:, jc0:jc0 + 2, :], pb2[:],
                                             Act.Relu, bias=nlam_a[:, 0:1],
                                             scale=C_INIT * S_A)

                # ---- phase B: 8 fp8 DoubleRow steps ----
                for step in range(NSTEPS):
                    last = step == NSTEPS - 1
                    ht8 = htp.tile([P, NDM, T], dt.float8e4, tag="ht8")
                    for dp in range(NDM // 2):
                        ph2 = psum.tile([P, 2, T], dt.float32, tag="mm")
                        for j2 in range(2):
                            dmc = dp * 2 + j2
                            for c in range(NDL // 2):
                                nc.tensor.matmul(
                                    ph2[:, j2, :],
                                    wt8[:, 2 * c:2 * c + 2,
                                        dmc * P:(dmc + 1) * P],
                                    a8[:, 2 * c:2 * c + 2, :],
                                    start=(c == 0),
                                    stop=(c == NDL // 2 - 1),
                                    perf_mode=DR)
                        nc.scalar.activation(ht8[:, 2 * dp:2 * dp + 2, :],
                                             ph2[:], Act.Copy,
                                             scale=S_H / (S_W * S_A))
                    # u' = 0.9u + bp - inv*py.  t = 0.9u + bp is
                    # precomputed 2-3 pairs AHEAD of the matmuls (v on
                    # DVE 4x-mode / ACT, add on POOL/DVE), so after each
                    # psum lands only one DVE stt + the ACT relu remain
                    # on the critical path.
                    tts = {}

                    def emit_vt(jp):
                        jc0 = 2 * jp
                        v2 = tmp.tile([P, 2, T], dt.bfloat16, tag="v",
                                      name=f"v_{jp}")
                        if jp % 8 in V_ACT:
                            nc.scalar.activation(v2[:], u[:, jc0:jc0 + 2, :],
                                                 Act.Copy, scale=0.9)
                        else:
                            nc.vector.tensor_scalar_mul(
                                v2[:], u[:, jc0:jc0 + 2, :], 0.9)
                        t2 = tmp.tile([P, 2, T], dt.bfloat16, tag="q",
                                      name=f"t_{jp}")
                        eng = nc.vector if jp % 8 in TT_DVE else nc.gpsimd
                        eng.tensor_tensor(t2[:], v2[:],
                                          bp[:, jc0:jc0 + 2, :], op=Alu.add)
                        tts[jp] = t2

                    for jp in range(NDL // 2):
                        if jp == 0:
                            emit_vt(0)
                            emit_vt(1)
                        if jp + 2 < NDL // 2:
                            emit_vt(jp + 2)
                        py2 = psum.tile([P, 2, T], dt.float32, tag="mm")
                        for j2 in range(2):
                            jc = 2 * jp + j2
                            for ki, kp in enumerate((0, 1, 2, 3)):
                                nc.tensor.matmul(
                                    py2[:, j2, :],
                                    w018[:, 2 * kp:2 * kp + 2,
                                         jc * P:(jc + 1) * P],
                                    ht8[:, 2 * kp:2 * kp + 2, :],
                                    start=(ki == 0), stop=False,
                                    perf_mode=DR)
                            # -g2*a correction via fp8 diag stationary
                            nc.tensor.matmul(
                                py2[:, j2, :], dd8[:, 2 * jc:2 * jc + 2, :],
                                a8[:, jc:jc + 2, :],
                                start=False, stop=True, perf_mode=DR)
                        jc0 = 2 * jp
                        nc.vector.scalar_tensor_tensor(
                            u[:, jc0:jc0 + 2, :], py2[:], -INV_SY,
                            tts.pop(jp)[:], op0=Alu.mult, op1=Alu.add)
                        if jp >= NDL // 2 - 2:
                            grps = [(jc0, 2)]          # tail: pairs
                        elif jp % 2 == 1:
                            grps = [(jc0 - 2, 4)]      # quad [jq, jq+4)
                        else:
                            grps = []
                        for jq, glen in grps:
                            if not last:
                                nc.scalar.activation(a8[:, jq:jq + glen, :],
                                                     u[:, jq:jq + glen, :],
                                                     Act.Relu,
                                                     bias=nlam_a[:, 0:1],
                                                     scale=S_A)
                            else:
                                # bf16 activations for the final matmul,
                                # stored into bp's buffer (bp is dead now)
                                nc.scalar.activation(bp[:, jq:jq + glen, :],
                                                     u[:, jq:jq + glen, :],
                                                     Act.Relu,
                                                     bias=nlam[:, 0:1])

                # ---- phase C: out = a @ W^T (bf16), [tok, dm] layout ----
                for nh in range(DM // T):
                    pcs = [psum.tile([P, 2, T], dt.float32, tag="mm",
                                     name=f"pc{s}_{blk}_{nh}")
                           for s in range((T // P) // 2)]
                    for piece in range(4):
                        wsc = wstr.tile([P, NDM, T], dt.bfloat16, tag="ws")
                        nc.sync.dma_start(
                            wsc[:], wtbf_d[:, piece * 8:piece * 8 + 8,
                                           nh * T:(nh + 1) * T])
                        for kl in range(8):
                            kc = piece * 8 + kl
                            for sub in range(T // P):
                                nc.tensor.matmul(
                                    pcs[sub // 2][:, sub % 2, :],
                                    bp[:, kc, sub * P:(sub + 1) * P],
                                    wsc[:, kl, :],
                                    start=(kc == 0), stop=(kc == NDL - 1))
                    for sp in range((T // P) // 2):
                        ob2 = oio.tile([P, 2, T], dt.float32, tag="ob")
                        nc.scalar.activation(ob2[:], pcs[sp][:], Act.Copy)
                        for j2 in range(2):
                            row = blk * T + (sp * 2 + j2) * P
                            nc.sync.dma_start(
                                out_d[row:row + P, nh * T:(nh + 1) * T],
                                ob2[:, j2, :])

    nc.compile()
    return nc


def _get_nc():
    if "nc" not in _CACHE:
        _CACHE["nc"] = _build_nc()
    return _CACHE["nc"]


def _prep_shared(W):
    W = np.asarray(W, np.float32)
    wt_l = np.ascontiguousarray(
        W.T.reshape(NDL, P, DM).transpose(1, 0, 2))           # [P, NDL, DM]
    w01_l = np.ascontiguousarray(
        (0.1 * W).reshape(NDM, P, DL).transpose(1, 0, 2))     # [P, NDM, DL]
    wt8 = (wt_l * S_W).astype(F8)
    w018 = (w01_l * S_W01).astype(F8)
    w01bf = w01_l.astype(BF16)
    wtbf = wt_l.astype(BF16)
    g2 = 0.1 * (W.astype(np.float64) ** 2).sum(0).astype(np.float32)
    dval = -(g2 * (S_W01 * S_H / S_A))                        # [DL]
    dd8 = np.zeros((P, 2 * NDL, P), np.float32)
    for j in range(NDL):
        np.fill_diagonal(dd8[:, 2 * j, :], dval[j * P:(j + 1) * P])
    dd8 = dd8.astype(F8)
    return wt8, w018, dd8, w01bf, wtbf


def make_in_maps(x, W):
    x = np.asarray(x)
    xf = x.reshape(-1, DM).astype(np.float32)
    wt8, w018, dd8, w01bf, wtbf = _prep_shared(W)

    in_maps = []
    for c in range(NCORES):
        xs = xf[c * TOK_CORE:(c + 1) * TOK_CORE]
        xt = np.ascontiguousarray(
            xs.reshape(NBLK, T, NDM, P).transpose(0, 3, 2, 1)).astype(BF16)
        in_maps.append({"xt": xt, "wt8": wt8, "w018": w018, "dd8": dd8,
                        "w01bf": w01bf, "wtbf": wtbf})
    return in_maps


def kernel(x, W):
    import os

    from concourse.bass_utils import run_bass_kernel_spmd

    if not TRACE:
        os.environ.setdefault("BASS_NEVER_TRACE", "1")
    x = np.asarray(x)
    orig_shape = x.shape
    in_maps = make_in_maps(x, W)

    nc = _get_nc()
    res = run_bass_kernel_spmd(nc, in_maps, core_ids=list(range(NCORES)),
                               trace=TRACE)
    global LAST_RESULT
    LAST_RESULT = res
    out = np.concatenate([res.results[c]["out"] for c in range(NCORES)], axis=0)
    return out.reshape(orig_shape).astype(np.float32)
